# revision 7
# baseline (speedup 1.0000x reference)
"""Trainium2 Bass kernel for nn_PoolNU: gather + max-pool over neighbour table.

reference:
    x: (8, 128, 65536) f32, neighbours: (9, 16384) int
    out[b, c, j] = max_k x[b, c, neighbours[k, j]]

Strategy:
    - Shard batch B=8 across the 8 NeuronCores (one batch row per core).
    - Host transposes x to (B, LIN, C) so one neighbour index addresses a
      contiguous 512B row (all 128 channels). The device gathers rows from
      HBM with the gpsimd dma_gather instruction and max-reduces over the
      K=9 gathered rows per output location on the vector engine.
    - dma_gather indices are int16 (rows 0..32767 per call), so the row space
      is split into two 32768-row windows. Each output location's 9 indices
      are sorted; output locations are grouped by how many of their indices
      fall in the low window (n). Tiles are n-pure, so every tile needs
      exactly one low-window gather of n rows/loc and one high-window gather
      of 9-n rows/loc — no wasted gather traffic. The host freely permutes
      the output-location axis (it already transposes the output) and
      unscrambles at the end.
"""

import sys

sys.path.insert(0, "/opt/trn_rl_repo")

import hashlib

import numpy as np

import concourse.mybir as mybir
from concourse import bacc, bass_utils
from concourse.tile import TileContext

B = 8
C = 128
LIN = 65536
K = 9
LOUT = 16384
HALF = 32768

P = 128
JMAX = 1024          # max output locations per tile

_CACHE = {}


def _make_plan(neighbours: np.ndarray):
    nb = np.asarray(neighbours).astype(np.int64)          # (K, LOUT)
    assert nb.shape == (K, LOUT)
    snb = np.sort(nb, axis=0)                             # per-j ascending
    n_j = (snb < HALF).sum(axis=0)                        # (LOUT,)

    tiles = []   # (locs (tsize,), S0)
    for n in range(K + 1):
        locs = np.where(n_j == n)[0]
        if len(locs) == 0:
            continue
        for start in range(0, len(locs), JMAX):
            chunk = locs[start : start + JMAX]
            pad = (-len(chunk)) % P
            if pad:
                chunk = np.concatenate([chunk, np.repeat(chunk[-1], pad)])
            tiles.append((chunk, n))

    # Each dma_gather call is capped at NMAX indices (HW descriptor-ring
    # limit). Calls cover contiguous runs of g-blocks (block = 128 rows of
    # 512B); block g = s*nslot + m of a tile.
    NMAX = 1024
    idx_blocks = []   # (16, w) int16 per call, in order
    meta = []         # per tile: dict with list of calls
    rows = []
    col = 0
    rowbase = 0
    for chunk, n in tiles:
        tsize = len(chunk)
        nslot = tsize // P
        locs2d = chunk.reshape(P, nslot)                  # [p, m]
        entry = {"tsize": tsize, "nslot": nslot, "S0": n, "S1": K - n,
                 "rowbase": rowbase, "calls": []}
        cs = max(1, (NMAX // P) // nslot)                 # s-values per call
        for lo in (True, False):
            S = n if lo else K - n
            if lo:
                vals = snb[:n, :][:, locs2d]              # (S, P, nslot)
            else:
                vals = snb[n:, :][:, locs2d] - HALF       # (S, P, nslot)
            for s0 in range(0, S, cs):
                sn = min(cs, S - s0)
                sub = vals[s0 : s0 + sn]                  # (sn, P, nslot)
                lst = sub.transpose(0, 2, 1).ravel()      # i=(s*nslot+m)*128+p
                nidx = len(lst)
                w = nidx // 16
                wrapped = lst.reshape(w, 16).T.astype(np.int16)
                idx_blocks.append(wrapped)
                goff = ((0 if lo else n) + s0) * nslot    # block offset in tile
                entry["calls"].append(
                    {"lo": lo, "col": col, "w": w, "nidx": nidx,
                     "goff": goff, "gn": sn * nslot}
                )
                col += w
        meta.append(entry)
        rows.append(chunk)
        rowbase += tsize

    idx16 = np.concatenate(idx_blocks, axis=1)            # (16, Wtot)
    idx_np = np.tile(idx16, (8, 1))                       # replicate across Q7 cores
    rows_all = np.concatenate(rows)                       # (ROWS_TOT,)
    return {"meta": meta, "idx": np.ascontiguousarray(idx_np),
            "rows": rows_all, "wtot": int(idx_np.shape[1]),
            "rows_tot": int(rowbase),
            "key": hashlib.sha1(nb.tobytes()).hexdigest()}


def _build_program(plan):
    wtot = plan["wtot"]
    rows_tot = plan["rows_tot"]

    nc = bacc.Bacc("TRN2", target_bir_lowering=False, debug=False, num_devices=1)
    xt = nc.dram_tensor("xt", [LIN, C], mybir.dt.float32, kind="ExternalInput")
    idx = nc.dram_tensor("idx", [P, wtot], mybir.dt.int16, kind="ExternalInput")
    out = nc.dram_tensor("out", [rows_tot, C], mybir.dt.float32,
                         kind="ExternalOutput")

    lo_ap = xt.ap()[0:HALF, :]
    hi_ap = xt.ap()[HALF:LIN, :]

    with TileContext(nc) as tc:
        with tc.tile_pool(name="sbuf", bufs=3) as pool:
            idx_sb = pool.tile([P, wtot], mybir.dt.int16, bufs=1)
            nc.sync.dma_start(out=idx_sb[:], in_=idx.ap())

            for t in plan["meta"]:
                nslot, tsize = t["nslot"], t["tsize"]
                g = pool.tile([P, JMAX // P * K * C], mybir.dt.float32, tag="g")
                gK = g[:, : nslot * K * C]
                for call in t["calls"]:
                    nc.gpsimd.dma_gather(
                        out_ap=gK[
                            :, call["goff"] * C : (call["goff"] + call["gn"]) * C
                        ].rearrange("p (g e) -> p g e", e=C),
                        in_ap=lo_ap if call["lo"] else hi_ap,
                        idxs_ap=idx_sb[:, call["col"] : call["col"] + call["w"]],
                        num_idxs=call["nidx"],
                        num_idxs_reg=call["nidx"],
                        elem_size=C,
                    )
                acc = pool.tile([P, JMAX // P * C], mybir.dt.float32,
                                tag="acc", bufs=3)
                accT = acc[:, : nslot * C]
                nc.vector.tensor_reduce(
                    out=accT,
                    in_=gK.rearrange("p (s m c) -> p m c s", s=K, m=nslot, c=C),
                    axis=mybir.AxisListType.X,
                    op=mybir.AluOpType.max,
                )
                dst = (
                    out.ap()[t["rowbase"] : t["rowbase"] + tsize, :]
                    .rearrange("(p m) c -> p (m c)", p=P)
                )
                nc.sync.dma_start(out=dst, in_=accT)

    nc.compile()
    return nc


def _get(neighbours: np.ndarray):
    plan = _make_plan(neighbours)
    key = plan["key"]
    if key not in _CACHE:
        _CACHE[key] = (_build_program(plan), plan)
    return _CACHE[key]


def kernel(x: np.ndarray, neighbours: np.ndarray) -> np.ndarray:
    x = np.asarray(x)
    assert x.shape == (B, C, LIN) and x.dtype == np.float32
    xt = np.ascontiguousarray(x.transpose(0, 2, 1))       # (B, LIN, C)

    nc, plan = _get(neighbours)
    in_maps = [{"xt": xt[b], "idx": plan["idx"]} for b in range(B)]
    res = bass_utils.run_bass_kernel_spmd(nc, in_maps, core_ids=list(range(B)))
    _CACHE["last_result"] = res

    rows = plan["rows"]
    outv = np.empty((B, C, LOUT), dtype=np.float32)
    for b in range(B):
        dev = res.results[b]["out"]                       # (ROWS_TOT, C)
        outv[b][:, rows] = dev.T
    return outv


# revision 8
# speedup vs baseline: 4.5757x; 4.5757x over previous
"""Trainium2 Bass kernel for nn_PoolNU: gather + max-pool over neighbour table.

reference:
    x: (8, 128, 65536) f32, neighbours: (9, 16384) int
    out[b, c, j] = max_k x[b, c, neighbours[k, j]]

Strategy:
    - The neighbour table is shared across (b, c), so one gathered "row" can
      carry ALL batches and channels for a location. Host repacks x to
      x_merged (65536, B*C=1024) — one 4KB row per location. This makes each
      gathered descriptor 4KB instead of 512B: 8x fewer descriptors, which
      matters because the gpsimd dma_gather ucode generates descriptors at
      only ~6-8 ns each.
    - Output locations (16384) are sharded across the 8 NeuronCores (2048
      per core). Each core needs at most 9*2048=18432 distinct source rows,
      which the host compacts into a per-core x_sub with remapped indices —
      guaranteed to fit dma_gather's int16 index window (< 32768), so no
      window splitting is needed at all.
    - Device per tile of 128 locations: gather 9*128 rows (two <=1024-index
      dma_gather calls), vector max-reduce over the 9 slots, store 4KB rows.
    - Host reassembles (core, loc, b, c) -> (b, c, loc).
"""

import sys

sys.path.insert(0, "/opt/trn_rl_repo")

import hashlib

import numpy as np

import concourse.mybir as mybir
from concourse import bacc, bass_utils
from concourse.tile import TileContext

B = 8
C = 128
LIN = 65536
K = 9
LOUT = 16384

P = 128
NCORE = 8
LPC = LOUT // NCORE          # locations per core (2048)
NTILE = LPC // P             # tiles per core (16)
E = B * C                    # elements per gathered row (1024)
UMAX = K * LPC               # padded x_sub rows (18432)
NMAX = 1024                  # max indices per dma_gather call

_CACHE = {}


def _build_program():
    nc = bacc.Bacc("TRN2", target_bir_lowering=False, debug=False, num_devices=1)

    xs = nc.dram_tensor("xs", [UMAX, E], mybir.dt.float32, kind="ExternalInput")
    # idx per tile: two calls (8 slots then 1 slot), each 16-wrapped and
    # replicated over the 128 partitions in groups of 16.
    WA, WB = NMAX // 16, P // 16
    WT = WA + WB
    idx = nc.dram_tensor("idx", [P, NTILE * WT], mybir.dt.int16,
                         kind="ExternalInput")
    out = nc.dram_tensor("out", [LPC, E], mybir.dt.float32, kind="ExternalOutput")

    with TileContext(nc) as tc:
        with tc.tile_pool(name="sbuf", bufs=3) as pool:
            idx_sb = pool.tile([P, NTILE * WT], mybir.dt.int16, bufs=1)
            nc.sync.dma_start(out=idx_sb[:], in_=idx.ap())

            for t in range(NTILE):
                g = pool.tile([P, K * E], mybir.dt.float32, tag="g")
                c0 = t * WT
                nc.gpsimd.dma_gather(
                    out_ap=g[:, : 8 * E].rearrange("p (g e) -> p g e", e=E),
                    in_ap=xs.ap(),
                    idxs_ap=idx_sb[:, c0 : c0 + WA],
                    num_idxs=NMAX,
                    num_idxs_reg=NMAX,
                    elem_size=E,
                )
                nc.gpsimd.dma_gather(
                    out_ap=g[:, 8 * E : K * E].rearrange("p (g e) -> p g e", e=E),
                    in_ap=xs.ap(),
                    idxs_ap=idx_sb[:, c0 + WA : c0 + WT],
                    num_idxs=P,
                    num_idxs_reg=P,
                    elem_size=E,
                )
                acc = pool.tile([P, E], mybir.dt.float32, tag="acc", bufs=3)
                nc.vector.tensor_reduce(
                    out=acc[:],
                    in_=g[:].rearrange("p (s e) -> p e s", s=K, e=E),
                    axis=mybir.AxisListType.X,
                    op=mybir.AluOpType.max,
                )
                nc.sync.dma_start(out=out.ap()[t * P : (t + 1) * P, :], in_=acc[:])

    nc.compile()
    return nc


def _get_program():
    if "nc" not in _CACHE:
        _CACHE["nc"] = _build_program()
    return _CACHE["nc"]


def _wrap16(lst: np.ndarray) -> np.ndarray:
    """(N,) int -> (128, N/16) int16: 16-partition wrap, replicated x8."""
    w = len(lst) // 16
    return np.tile(lst.reshape(w, 16).T, (8, 1)).astype(np.int16)


def kernel(x: np.ndarray, neighbours: np.ndarray) -> np.ndarray:
    x = np.asarray(x)
    nb = np.asarray(neighbours).astype(np.int64)          # (K, LOUT)
    assert x.shape == (B, C, LIN) and x.dtype == np.float32
    assert nb.shape == (K, LOUT)

    # (LIN, B*C): one 4KB row per input location
    xm = np.ascontiguousarray(x.transpose(2, 0, 1).reshape(LIN, E))

    in_maps = []
    for core in range(NCORE):
        nbc = nb[:, core * LPC : (core + 1) * LPC]        # (K, LPC)
        uniq, inv = np.unique(nbc, return_inverse=True)
        inv = inv.reshape(K, LPC)
        xs = np.empty((UMAX, E), dtype=np.float32)
        xs[: len(uniq)] = xm[uniq]
        cols = []
        for t in range(NTILE):
            loc2d = inv[:, t * P : (t + 1) * P]           # (K, P) local idx
            # call A: slots 0..7 -> list[(s)*128+p] = loc2d[s, p]
            cols.append(_wrap16(loc2d[:8].ravel()))
            # call B: slot 8
            cols.append(_wrap16(loc2d[8].ravel()))
        idx_np = np.ascontiguousarray(np.concatenate(cols, axis=1))
        in_maps.append({"xs": xs, "idx": idx_np})

    nc = _get_program()
    res = bass_utils.run_bass_kernel_spmd(nc, in_maps, core_ids=list(range(NCORE)))
    _CACHE["last_result"] = res

    # out per core: (LPC, B*C) -> full (B, C, LOUT)
    dev = np.concatenate([res.results[c]["out"] for c in range(NCORE)])  # (LOUT, E)
    return np.ascontiguousarray(dev.reshape(LOUT, B, C).transpose(1, 2, 0))
